# revision 9
# baseline (speedup 1.0000x reference)
"""Trainium2 Bass kernel for the GNN message-passing layer.

Reference computation (per node n, K=32 neighbors, 128-d features):
    h_c    = h_center @ W_i + b_i                        [N, 128]
    filt   = einsum('nkf,fo->nko', diff_features, W_gamma)
    msg    = sum_k(filt * h_neighbors)                   [N, 128]
    out    = silu(h_c + msg)

Sharding: node axis split across 8 NeuronCores (data parallel), weights
replicated. Each core processes 6272 padded nodes (6250 real).

Per-core layout ("macro tile" = 512 edge-rows = 16 nodes x 32 k):
  - diff rows are packed 4/partition: SBUF tile [128, 128] where
    partition p holds rows 4p..4p+3 (f-major). One PE transpose yields
    diffT[32j+f, p] = diff[row 4p+j, f].
  - filter: ONE K=128 matmul per macro with lhsT = diffT and a
    block-diagonal replicated W_gamma [128, 512] as the moving operand:
    f_ps[p, 128j+o] = filt[row 4p+j, o].
  - h_neighbors macro tile [128, 512] loads fully contiguous and matches
    that layout exactly; one DVE tensor_mul forms A = filt * h_nb.
  - k-reduce: node(4p+j) = p//8 (independent of j), so a constant
    0/1 matrix ones_u [128, 128] (columns 16u..16u+16 hold the node map)
    contracts partitions in ONE N=512 matmul per macro, accumulating
    j-separated partial messages in PSUM [128, 512] across the 8 macros
    of a 128-node group; two DVE adds fold the four j-blocks.
  - h_center @ W_i (via PE transpose of h_center) seeds the same PSUM
    bank (start=True); bias added with a K=1 ones x b_i matmul.
  - silu = ScalarE Sigmoid out of PSUM + DVE multiply; contiguous store.
"""

import sys

if "/opt/trn_rl_repo" not in sys.path:
    sys.path.insert(0, "/opt/trn_rl_repo")

import numpy as np

N_TOTAL = 50000
K = 32
D_IN = 128
F_DIM = 32
OUT_DIM = 128
N_CORES = 8

NODES_PER_CORE = 6272          # padded: 49 groups of 128 nodes
N_PADDED = NODES_PER_CORE * N_CORES
GROUPS = NODES_PER_CORE // 128             # 49
MACROS_PER_GROUP = 8                       # 16 nodes per macro

_CACHE = {}
_TRACE = False
_LAST_RESULTS = [None]


def _split_sync_waits(nc, max_waits=1):
    """walrus in this container accepts only one sem-wait per instruction;
    peel extra waits onto chained NoOps on the same engine."""
    import concourse.mybir as mybir

    n_fix = 0
    for fn in nc.m.functions:
        for blk in fn.blocks:
            insts = blk.instructions
            i = 0
            while i < len(insts):
                inst = insts[i]
                si = getattr(inst, "sync_info", None)
                keep_n = 0 if type(inst).__name__ == "InstDrain" else max_waits
                if si is not None and si.on_wait and len(si.on_wait) > keep_n:
                    waits = list(si.on_wait)
                    keep, rest = waits[:keep_n], waits[keep_n:]
                    si.on_wait = keep
                    new_insts = []
                    while rest:
                        chunk, rest = rest[:max_waits], rest[max_waits:]
                        n_fix += 1
                        new_insts.append(
                            mybir.InstNoOp(
                                name=f"waitfix-{n_fix}-{inst.name}",
                                engine=inst.engine,
                                bass_nofuse=True,
                                sync_info=mybir.SyncInfo(on_wait=chunk, on_update=[]),
                            )
                        )
                    insts[i:i] = new_insts
                    i += len(new_insts)
                i += 1
    return n_fix


def _emit(tc, io, groups):
    """Emit the per-core kernel body. io: dict of APs."""
    import concourse.mybir as mybir

    f32 = mybir.dt.float32
    nc = tc.nc

    hnb_v = io["h_neighbors"].rearrange("(t x) k o -> t (x k o)", x=16).rearrange(
        "t (p c) -> t p c", p=128)
    diff_v = io["diff_features"].rearrange("(t x) k f -> t (x k f)", x=16).rearrange(
        "t (p c) -> t p c", p=128)
    hc_v = io["h_center"].rearrange("(g p) d -> g p d", p=128)
    out_v = io["out"].rearrange("(g p) d -> g p d", p=128)

    with (
        tc.tile_pool(name="consts", bufs=1) as consts,
        tc.tile_pool(name="hnb", bufs=6) as hnb_pool,
        tc.tile_pool(name="diff", bufs=6) as diff_pool,
        tc.tile_pool(name="diffT", bufs=4) as diffT_pool,
        tc.tile_pool(name="amul", bufs=4) as a_pool,
        tc.tile_pool(name="hc", bufs=2) as hc_pool,
        tc.tile_pool(name="hcT", bufs=2) as hcT_pool,
        tc.tile_pool(name="outp", bufs=3) as out_pool,
        tc.tile_pool(name="ps_t", bufs=2, space="PSUM") as ps_t,
        tc.tile_pool(name="ps_f", bufs=2, space="PSUM") as ps_f,
        tc.tile_pool(name="ps_m", bufs=2, space="PSUM") as ps_m,
    ):
        ident = consts.tile([128, 128], f32)
        nc.sync.dma_start(out=ident, in_=io["ident"])
        w_bd = consts.tile([128, 4 * OUT_DIM], f32)
        nc.sync.dma_start(out=w_bd, in_=io["w_bd"])
        w_i = consts.tile([D_IN, 4 * OUT_DIM], f32)
        nc.sync.dma_start(out=w_i, in_=io["w_i"])
        b_i = consts.tile([1, 4 * OUT_DIM], f32)
        nc.sync.dma_start(out=b_i, in_=io["b_i"])
        ones1 = consts.tile([1, 128], f32)
        nc.sync.dma_start(out=ones1, in_=io["ones1"])
        ones_all = consts.tile([128, 8 * 128], f32)
        nc.sync.dma_start(out=ones_all, in_=io["ones_all"])

        for g in range(groups):
            hc_sb = hc_pool.tile([128, 128], f32)
            nc.sync.dma_start(out=hc_sb, in_=hc_v[g])
            hcT_ps = ps_t.tile([128, 128], f32, tag="tp")
            nc.tensor.transpose(hcT_ps, hc_sb, ident)
            hcT_sb = hcT_pool.tile([128, 128], f32)
            nc.scalar.copy(hcT_sb, hcT_ps)

            msg_ps = ps_m.tile([128, 4 * OUT_DIM], f32)
            # h_center @ [W_i | 0 0 0] seeds the full accumulator bank
            nc.tensor.matmul(msg_ps, lhsT=hcT_sb, rhs=w_i,
                             start=True, stop=False, skip_group_check=True)
            # + [b_i | 0 0 0] broadcast over the 128 nodes (K=1 matmul)
            nc.tensor.matmul(msg_ps, lhsT=ones1, rhs=b_i,
                             start=False, stop=False, skip_group_check=True)

            for u in range(MACROS_PER_GROUP):
                t = MACROS_PER_GROUP * g + u
                diff_sb = diff_pool.tile([128, 128], f32)
                nc.sync.dma_start(out=diff_sb, in_=diff_v[t])
                hnb_sb = hnb_pool.tile([128, 4 * OUT_DIM], f32)
                nc.sync.dma_start(out=hnb_sb, in_=hnb_v[t])

                dT_ps = ps_t.tile([128, 128], f32, tag="tp")
                nc.tensor.transpose(dT_ps, diff_sb, ident)
                dT_sb = diffT_pool.tile([128, 128], f32)
                nc.scalar.copy(dT_sb, dT_ps)

                f_ps = ps_f.tile([128, 4 * OUT_DIM], f32)
                nc.tensor.matmul(f_ps, lhsT=dT_sb, rhs=w_bd,
                                 start=True, stop=True)

                a_sb = a_pool.tile([128, 4 * OUT_DIM], f32)
                nc.vector.tensor_mul(a_sb, f_ps, hnb_sb)

                nc.tensor.matmul(
                    msg_ps,
                    lhsT=ones_all[:, 128 * u:128 * (u + 1)],
                    rhs=a_sb,
                    start=False,
                    stop=(u == MACROS_PER_GROUP - 1),
                    skip_group_check=True,
                )

            # fold the four j-blocks (DVE can read at most one PSUM input)
            c23_sb = out_pool.tile([128, 2 * OUT_DIM], f32, tag="c23")
            nc.scalar.copy(c23_sb, msg_ps[:, 256:512])
            t_sb = out_pool.tile([128, 2 * OUT_DIM], f32, tag="tfold")
            nc.vector.tensor_add(t_sb, msg_ps[:, 0:256], c23_sb)
            m_sb = out_pool.tile([128, OUT_DIM], f32, tag="mfold")
            nc.vector.tensor_add(m_sb, t_sb[:, 0:OUT_DIM],
                                 t_sb[:, OUT_DIM:2 * OUT_DIM])

            sig_sb = out_pool.tile([128, OUT_DIM], f32, tag="sig")
            nc.scalar.activation(sig_sb, m_sb,
                                 mybir.ActivationFunctionType.Sigmoid)
            out_sb = out_pool.tile([128, OUT_DIM], f32, tag="out")
            nc.vector.tensor_mul(out_sb, sig_sb, m_sb)
            nc.sync.dma_start(out=out_v[g], in_=out_sb)


def _build_bass(groups=GROUPS):
    import concourse.bass as bass
    import concourse.mybir as mybir
    import concourse.tile as tile

    f32 = mybir.dt.float32
    nc = bass.Bass()
    nodes = groups * 128

    io = {}
    io["h_center"] = nc.dram_tensor(
        "h_center", [nodes, D_IN], f32, kind="ExternalInput")[:]
    io["h_neighbors"] = nc.dram_tensor(
        "h_neighbors", [nodes, K, OUT_DIM], f32, kind="ExternalInput")[:]
    io["diff_features"] = nc.dram_tensor(
        "diff_features", [nodes, K, F_DIM], f32, kind="ExternalInput")[:]
    io["ident"] = nc.dram_tensor("ident", [128, 128], f32,
                                 kind="ExternalInput")[:]
    io["w_bd"] = nc.dram_tensor("w_bd", [128, 4 * OUT_DIM], f32,
                                kind="ExternalInput")[:]
    io["w_i"] = nc.dram_tensor("w_i", [D_IN, 4 * OUT_DIM], f32,
                               kind="ExternalInput")[:]
    io["b_i"] = nc.dram_tensor("b_i", [1, 4 * OUT_DIM], f32,
                               kind="ExternalInput")[:]
    io["ones1"] = nc.dram_tensor("ones1", [1, 128], f32,
                                 kind="ExternalInput")[:]
    io["ones_all"] = nc.dram_tensor("ones_all", [128, 8 * 128], f32,
                                    kind="ExternalInput")[:]
    io["out"] = nc.dram_tensor("out", [nodes, OUT_DIM], f32,
                               kind="ExternalOutput")[:]

    with tile.TileContext(nc) as tc:
        _emit(tc, io, groups)

    _split_sync_waits(nc)
    return nc


def make_consts(w_gamma):
    w_gamma = np.asarray(w_gamma, dtype=np.float32)
    ident = np.eye(128, dtype=np.float32)
    ones1 = np.ones((1, 128), dtype=np.float32)
    # block-diagonal W_gamma: w_bd[32j'+f, 128j+o] = W_gamma[f,o] * (j==j')
    w_bd = np.zeros((128, 4 * OUT_DIM), dtype=np.float32)
    for j in range(4):
        w_bd[32 * j:32 * (j + 1), 128 * j:128 * (j + 1)] = w_gamma
    # ones_all[p, 128u + m] = 1 iff m == 16u + p//8
    p = np.arange(128)
    ones_all = np.zeros((128, 8 * 128), dtype=np.float32)
    for u in range(8):
        ones_all[p, 128 * u + 16 * u + p // 8] = 1.0
    return {"ident": ident, "w_bd": w_bd, "ones1": ones1,
            "ones_all": ones_all}


def kernel(h_center, h_neighbors, diff_features, W_i, b_i, W_gamma):
    from concourse.bass_utils import run_bass_kernel_spmd

    h_center = np.ascontiguousarray(h_center, dtype=np.float32)
    h_neighbors = np.ascontiguousarray(h_neighbors, dtype=np.float32)
    diff_features = np.ascontiguousarray(diff_features, dtype=np.float32)
    W_i = np.ascontiguousarray(W_i, dtype=np.float32)
    w_i_wide = np.zeros((D_IN, 4 * OUT_DIM), dtype=np.float32)
    w_i_wide[:, :OUT_DIM] = W_i
    b_i_row = np.zeros((1, 4 * OUT_DIM), dtype=np.float32)
    b_i_row[0, :OUT_DIM] = np.asarray(b_i, dtype=np.float32).reshape(-1)
    W_gamma = np.ascontiguousarray(W_gamma, dtype=np.float32)

    n = h_center.shape[0]
    pad = N_PADDED - n

    def pad0(a):
        if pad == 0:
            return a
        return np.concatenate(
            [a, np.zeros((pad,) + a.shape[1:], dtype=a.dtype)], axis=0)

    h_center_p = pad0(h_center)
    h_nb_p = pad0(h_neighbors)
    diff_p = pad0(diff_features)

    consts = make_consts(W_gamma)

    if "nc" not in _CACHE:
        _CACHE["nc"] = _build_bass()
    nc = _CACHE["nc"]

    in_maps = []
    for i in range(N_CORES):
        s = slice(i * NODES_PER_CORE, (i + 1) * NODES_PER_CORE)
        in_maps.append({
            "h_center": h_center_p[s],
            "h_neighbors": h_nb_p[s],
            "diff_features": diff_p[s],
            "w_i": w_i_wide,
            "b_i": b_i_row,
            **consts,
        })

    res = run_bass_kernel_spmd(nc, in_maps, core_ids=list(range(N_CORES)),
                               trace=_TRACE)
    _LAST_RESULTS[0] = res
    full = np.concatenate([res.results[i]["out"] for i in range(N_CORES)],
                          axis=0)
    return full[:n]
